# revision 3
# baseline (speedup 1.0000x reference)
"""Segment-sharded multi-head attention (GNN message passing) for 8 trn2 cores.

Problem: B=64 global queries, E=65536 edges, N2=256, H=8 heads.
reference returns (x [B,N2], attn [H,B,E]); attn is ~98.5% structural zeros
(each edge only attends within its own segment b=batch[e]).

Sharding strategy (per the "shard by segment" hint): sort edges by batch id
on the host (that IS the sharding step), give each core 8 consecutive
segments (~8192 edges). Every softmax row (h, b) is then fully local to one
core -> no cross-device reduction at all, and each core computes only the
nonzero [64 x ~8192] block of attn.

Device kernel per core (CAP = padded edge count):
  scoresT[e, hb] = sum_d key[e,d] * A[hb,d]      (A = q @ Wk_head, host-folded)
  explocal = exp(scoresT) * segmask              (softmax max-shift cancels;
                                                  scores are O(30) so exp is
                                                  safe in f32 without it)
  U_unnorm[hb, 0:256] += explocal.T @ value      (PE accumulation)
  U_unnorm[hb, 256]   += sum_e explocal          (ones column of value)
Host epilogue: attn = explocal / gsum scattered to original edge order,
x = rowwise (U/gsum) @ Wv_h.T (+bv) concat-heads @ Wo.T + bo.

DMA-layout notes: value / batch / attn_un use a partition-major DRAM
packing [128, NT, ...] (edge n*128+p lives at [p, n]) so every DMA burst
is >=2KB contiguous per partition; the segment mask is built on device
from batch ids (is_equal against a per-column b-value table).
"""

import numpy as np

import concourse.bacc as bacc
import concourse.mybir as mybir
import concourse.tile as tile
from concourse.bass_utils import run_bass_kernel_spmd

B = 64          # segments / queries
E = 65536       # edges
H = 8           # heads
DK = 32         # head dim
N2 = 256        # model dim
NC = 8          # cores
SPC = 8         # segments per core
HB = 64         # rows per core = H * SPC
SUP = 1024      # edges per super-tile

F32 = mybir.dt.float32

_kernel_cache: dict[int, object] = {}


def _build_kernel(cap: int):
    """Bass kernel for one core; cap must be a multiple of SUP."""
    assert cap % SUP == 0
    ST = cap // SUP
    NT = cap // 128

    nc = bacc.Bacc("TRN2", target_bir_lowering=False, debug=False,
                   num_devices=NC)

    kt = nc.dram_tensor("kt", [N2, cap], F32, kind="ExternalInput")
    valp = nc.dram_tensor("valp", [128, NT, N2], F32, kind="ExternalInput")
    batchf = nc.dram_tensor("batchf", [128, NT], F32, kind="ExternalInput")
    bvals = nc.dram_tensor("bvals", [128, HB], F32, kind="ExternalInput")
    at = nc.dram_tensor("at", [N2, HB], F32, kind="ExternalInput")
    attn_un = nc.dram_tensor("attn_un", [128, NT, HB], F32,
                             kind="ExternalOutput")
    u_out = nc.dram_tensor("u_out", [HB, N2 + 1], F32, kind="ExternalOutput")

    with tile.TileContext(nc) as tc:
        with (
            tc.tile_pool(name="const", bufs=1) as constp,
            tc.tile_pool(name="ktp", bufs=3) as ktp,
            tc.tile_pool(name="valp_", bufs=3) as valpp,
            tc.tile_pool(name="mskp", bufs=3) as mskp,
            tc.tile_pool(name="expp", bufs=3) as expp,
            tc.tile_pool(name="psp", bufs=3, space="PSUM") as psp,
            tc.tile_pool(name="psu", bufs=1, space="PSUM") as psup,
        ):
            # Constants: A^T (two 128-row contraction chunks), b-value
            # table, per-edge batch ids.
            at_t = constp.tile([128, 2, HB], F32)
            nc.sync.dma_start(
                at_t[:], at[:, :].rearrange("(c p) h -> p c h", p=128))
            bv_t = constp.tile([128, HB], F32)
            nc.sync.dma_start(bv_t[:], bvals[:, :])
            bat_t = constp.tile([128, NT], F32)
            nc.sync.dma_start(bat_t[:], batchf[:, :])

            u_ps = psup.tile([HB, N2 + 1], F32)

            for s in range(ST):
                esl = slice(s * SUP, (s + 1) * SUP)
                tsl = slice(s * 8, (s + 1) * 8)

                kt_t = ktp.tile([128, 2, SUP], F32)
                nc.sync.dma_start(
                    kt_t[:], kt[:, esl].rearrange("(c p) e -> p c e", p=128))

                val_t = valpp.tile([128, 8, N2 + 1], F32)
                nc.sync.dma_start(val_t[:, :, 0:N2], valp[:, tsl, :])
                nc.vector.memset(val_t[:, :, N2], 1.0)

                # Segment mask from batch ids: msk[p,t,hb] =
                #   (bvals[hb] == batch[edge t*128+p])
                msk_t = mskp.tile([128, 8, HB], F32)
                for t in range(8):
                    col = s * 8 + t
                    nc.vector.tensor_scalar(
                        msk_t[:, t, :], bv_t[:], bat_t[:, col:col + 1], None,
                        mybir.AluOpType.is_equal)

                # scoresT for 8 sub-tiles of 128 edges, packed in free dim.
                ps_t = psp.tile([128, 8, HB], F32)
                for t in range(8):
                    ksl = slice(t * 128, (t + 1) * 128)
                    nc.tensor.matmul(ps_t[:, t, :], kt_t[:, 0, ksl],
                                     at_t[:, 0, :], start=True, stop=False,
                                     skip_group_check=True)
                    nc.tensor.matmul(ps_t[:, t, :], kt_t[:, 1, ksl],
                                     at_t[:, 1, :], start=False, stop=True,
                                     skip_group_check=True)

                exp_t = expp.tile([128, 8, HB], F32)
                nc.scalar.activation(exp_t[:], ps_t[:],
                                     mybir.ActivationFunctionType.Exp)
                nc.vector.tensor_mul(exp_t[:], exp_t[:], msk_t[:])

                for t in range(8):
                    nc.tensor.matmul(u_ps[:], exp_t[:, t, :], val_t[:, t, :],
                                     start=(s == 0 and t == 0),
                                     stop=(s == ST - 1 and t == 7),
                                     skip_group_check=True)

                nc.sync.dma_start(attn_un[:, tsl, :], exp_t[:])

            u_sb = constp.tile([HB, N2 + 1], F32)
            nc.vector.tensor_copy(u_sb[:], u_ps[:])
            nc.sync.dma_start(u_out[:], u_sb[:])

    nc.compile()
    return nc


def _get_kernel(cap: int):
    if cap not in _kernel_cache:
        _kernel_cache[cap] = _build_kernel(cap)
    return _kernel_cache[cap]


def _prep(global_query, local_key, local_value, batch,
          Wq, bq, Wk, bk, Wv, bv, Wo, bo):
    order = np.argsort(batch, kind="stable")
    sb = batch[order]
    counts = np.bincount(batch, minlength=B)
    lo = np.zeros(B + 1, np.int64)
    lo[1:] = np.cumsum(counts)
    core_lo = lo[::SPC]                       # [NC+1]
    cnts = np.diff(core_lo)
    cap = int(np.ceil(max(int(cnts.max()), SUP) / float(SUP)) * SUP)
    NT = cap // 128

    q = global_query.astype(np.float64) @ Wq.T.astype(np.float64) + bq
    # A[h, b, d] = q_h[b] . Wk_h[:, d]   (bk dropped: per-row shift cancels
    # in softmax)
    A = np.einsum("bhk,hkd->hbd", q.reshape(B, H, DK),
                  Wk.reshape(H, DK, N2).astype(np.float64)).astype(np.float32)

    keyT = np.ascontiguousarray(local_key[order].T)   # [N2, E]
    vals = local_value[order]                         # [E, N2]
    sbf = sb.astype(np.float32)

    in_maps = []
    for c in range(NC):
        s, e = int(core_lo[c]), int(core_lo[c + 1])
        n = e - s
        ktc = np.zeros((N2, cap), np.float32)
        ktc[:, :n] = keyT[:, s:e]
        vc = np.zeros((cap, N2), np.float32)
        vc[:n] = vals[s:e]
        vcp = np.ascontiguousarray(
            vc.reshape(NT, 128, N2).transpose(1, 0, 2))
        bc = np.full(cap, -1.0, np.float32)
        bc[:n] = sbf[s:e]
        bcp = np.ascontiguousarray(bc.reshape(NT, 128).T)
        segs = np.arange(c * SPC, (c + 1) * SPC, dtype=np.float32)
        bvrow = np.tile(segs, H)              # [HB], col h*SPC+j
        bvc = np.ascontiguousarray(
            np.broadcast_to(bvrow[None, :], (128, HB)))
        Ac = A[:, c * SPC:(c + 1) * SPC, :]   # [H, SPC, N2]
        atc = np.ascontiguousarray(Ac.reshape(HB, N2).T).astype(np.float32)
        in_maps.append({"kt": ktc, "valp": vcp, "batchf": bcp,
                        "bvals": bvc, "at": atc})

    return in_maps, order, lo, core_lo, cap


def kernel(**inputs):
    gq = np.asarray(inputs["global_query"], np.float32)
    key = np.asarray(inputs["local_key"], np.float32)
    value = np.asarray(inputs["local_value"], np.float32)
    batch = np.asarray(inputs["batch"])
    Wq = np.asarray(inputs["Wq"], np.float32)
    bq = np.asarray(inputs["bq"], np.float32)
    Wk = np.asarray(inputs["Wk"], np.float32)
    bk = np.asarray(inputs["bk"], np.float32)
    Wv = np.asarray(inputs["Wv"], np.float32)
    bv = np.asarray(inputs["bv"], np.float32)
    Wo = np.asarray(inputs["Wo"], np.float32)
    bo = np.asarray(inputs["bo"], np.float32)

    in_maps, order, lo, core_lo, cap = _prep(
        gq, key, value, batch, Wq, bq, Wk, bk, Wv, bv, Wo, bo)
    NT = cap // 128

    nc = _get_kernel(cap)
    res = run_bass_kernel_spmd(nc, in_maps, list(range(NC))).results

    attn = np.zeros((H, B, E), np.float32)
    xcat = np.zeros((B, N2), np.float32)
    WvH = Wv.reshape(H, DK, N2)
    bvH = bv.reshape(H, DK)

    for c in range(NC):
        aup = res[c]["attn_un"]               # [128, NT, HB]
        au = aup.transpose(1, 0, 2).reshape(cap, HB)
        u = res[c]["u_out"]                   # [HB, N2+1]
        gsum = u[:, N2]                       # [HB]
        nz = (gsum > 0).astype(np.float32)
        gsafe = np.where(gsum > 0, gsum, 1.0)
        Un = u[:, :N2] / gsafe[:, None]       # [HB, N2]

        s0 = int(core_lo[c])
        for j in range(SPC):
            b = c * SPC + j
            gs, ge = int(lo[b]), int(lo[b + 1])
            if ge == gs:
                continue
            eidx = order[gs:ge]
            rows = slice(gs - s0, ge - s0)
            valsb = au[rows, j::SPC]          # [cnt_b, H]
            inv = 1.0 / gsafe[j::SPC]         # [H]
            attn[:, b, eidx] = (valsb * inv[None, :]).T

        # out[h, j, dk] = Un[h*SPC+j] @ Wv_h.T + bv_h * (gsum>0)
        UnH = Un.reshape(H, SPC, N2)
        nzH = nz.reshape(H, SPC)
        o = np.einsum("hjd,hkd->hjk", UnH, WvH) + \
            bvH[:, None, :] * nzH[:, :, None]
        xcat[c * SPC:(c + 1) * SPC] = o.transpose(1, 0, 2).reshape(SPC, N2)

    x = (xcat @ Wo.T + bo).astype(np.float32)
    return x, attn


# revision 8
# speedup vs baseline: 1.2143x; 1.2143x over previous
"""Segment-sharded multi-head attention (GNN message passing) for 8 trn2 cores.

Problem: B=64 global queries, E=65536 edges, N2=256, H=8 heads.
reference returns (x [B,N2], attn [H,B,E]); attn is ~98.5% structural zeros
(each edge only attends within its own segment b=batch[e]).

Sharding strategy (per the "shard by segment" hint): sort edges by batch id
on the host (that IS the sharding step), give each core 8 consecutive
segments (~8192 edges). Every softmax row (h, b) is then fully local to one
core -> no cross-device reduction at all, and each core computes only the
nonzero [64 x ~8192] block of attn.

Device kernel per core (CAP = padded edge count):
  scoresT[e, hb] = sum_d key[e,d] * A[hb,d]      (A = q @ Wk_head, host-folded)
  explocal = exp(scoresT) * segmask              (softmax max-shift cancels;
                                                  scores are O(30) so exp is
                                                  safe in f32 without it)
  U_unnorm[hb, 0:256] += explocal.T @ value      (PE accumulation)
  U_unnorm[hb, 256]   += sum_e explocal          (ones column of value)
Host epilogue: attn = explocal / gsum scattered to original edge order,
x = rowwise (U/gsum) @ Wv_h.T (+bv) concat-heads @ Wo.T + bo.

DMA-layout notes: value / batch / attn_un use a partition-major DRAM
packing [128, NT, ...] (edge n*128+p lives at [p, n]) so every DMA burst
is >=2KB contiguous per partition; the segment mask is built on device
from batch ids (is_equal against a per-column b-value table).
"""

import numpy as np

import concourse.bacc as bacc
import concourse.mybir as mybir
import concourse.tile as tile
from concourse.bass_utils import run_bass_kernel_spmd

B = 64          # segments / queries
E = 65536       # edges
H = 8           # heads
DK = 32         # head dim
N2 = 256        # model dim
NC = 8          # cores
SPC = 8         # segments per core
HB = 64         # rows per core = H * SPC
SUP = 1024      # edges per super-tile

F32 = mybir.dt.float32
F32R = mybir.dt.float32r

_kernel_cache: dict[int, object] = {}


def _build_kernel(cap: int):
    """Bass kernel for one core; cap must be a multiple of SUP."""
    assert cap % SUP == 0
    ST = cap // SUP
    NT = cap // 128

    nc = bacc.Bacc("TRN2", target_bir_lowering=False, debug=False,
                   num_devices=NC)

    kt = nc.dram_tensor("kt", [N2, cap], F32R, kind="ExternalInput")
    valp = nc.dram_tensor("valp", [128, NT, N2 + 2], F32R,
                          kind="ExternalInput")
    batchf = nc.dram_tensor("batchf", [128, NT], F32, kind="ExternalInput")
    bvals = nc.dram_tensor("bvals", [128, HB], F32, kind="ExternalInput")
    at = nc.dram_tensor("at", [N2, HB], F32R, kind="ExternalInput")
    attn_un = nc.dram_tensor("attn_un", [128, NT, HB], F32R,
                             kind="ExternalOutput")
    u_out = nc.dram_tensor("u_out", [HB, N2 + 2], F32,
                            kind="ExternalOutput")

    with tile.TileContext(nc) as tc:
        with (
            tc.tile_pool(name="const", bufs=1) as constp,
            tc.tile_pool(name="ktp", bufs=3) as ktp,
            tc.tile_pool(name="valp_", bufs=3) as valpp,
            tc.tile_pool(name="mskp", bufs=3) as mskp,
            tc.tile_pool(name="expp", bufs=3) as expp,
            tc.tile_pool(name="psp", bufs=3, space="PSUM") as psp,
            tc.tile_pool(name="psu", bufs=1, space="PSUM") as psup,
        ):
            # Constants: A^T (two 128-row contraction chunks), b-value
            # table, per-edge batch ids.
            at_t = constp.tile([128, 2, HB], F32R)
            nc.sync.dma_start(
                at_t[:], at[:, :].rearrange("(c p) h -> p c h", p=128))
            bv_t = constp.tile([128, HB], F32)
            nc.sync.dma_start(bv_t[:], bvals[:, :])
            bat_t = constp.tile([128, NT], F32)
            nc.sync.dma_start(bat_t[:], batchf[:, :])

            u_ps = psup.tile([HB, N2 + 2], F32)

            for s in range(ST):
                esl = slice(s * SUP, (s + 1) * SUP)
                tsl = slice(s * 8, (s + 1) * 8)

                kt_t = ktp.tile([128, 2, SUP], F32R)
                nc.sync.dma_start(
                    kt_t[:], kt[:, esl].rearrange("(c p) e -> p c e", p=128))

                val_t = valpp.tile([128, 8, N2 + 2], F32R)
                nc.sync.dma_start(val_t[:], valp[:, tsl, :])

                # Segment mask from batch ids: msk[p,t,hb] =
                #   (bvals[hb] == batch[edge t*128+p])
                msk_t = mskp.tile([128, 8, HB], F32R)
                for t in range(8):
                    col = s * 8 + t
                    nc.vector.tensor_scalar(
                        msk_t[:, t, :], bv_t[:], bat_t[:, col:col + 1], None,
                        mybir.AluOpType.is_equal)

                # scoresT for 8 sub-tiles of 128 edges, packed in free dim.
                ps_t = psp.tile([128, 8, HB], F32)
                for t in range(8):
                    ksl = slice(t * 128, (t + 1) * 128)
                    nc.tensor.matmul(ps_t[:, t, :], kt_t[:, 0, ksl],
                                     at_t[:, 0, :], start=True, stop=False,
                                     skip_group_check=True)
                    nc.tensor.matmul(ps_t[:, t, :], kt_t[:, 1, ksl],
                                     at_t[:, 1, :], start=False, stop=True,
                                     skip_group_check=True)

                exp_t = expp.tile([128, 8, HB], F32R)
                nc.scalar.activation(exp_t[:], ps_t[:],
                                     mybir.ActivationFunctionType.Exp)
                nc.vector.tensor_mul(exp_t[:], exp_t[:], msk_t[:])

                for t in range(8):
                    nc.tensor.matmul(u_ps[:], exp_t[:, t, :], val_t[:, t, :],
                                     start=(s == 0 and t == 0),
                                     stop=(s == ST - 1 and t == 7),
                                     skip_group_check=True)

                nc.sync.dma_start(attn_un[:, tsl, :], exp_t[:])

            u_sb = constp.tile([HB, N2 + 2], F32)
            nc.vector.tensor_copy(u_sb[:], u_ps[:])
            nc.sync.dma_start(u_out[:], u_sb[:])

    nc.compile()
    return nc


def _get_kernel(cap: int):
    if cap not in _kernel_cache:
        _kernel_cache[cap] = _build_kernel(cap)
    return _kernel_cache[cap]


def _prep(global_query, local_key, local_value, batch,
          Wq, bq, Wk, bk, Wv, bv, Wo, bo):
    order = np.argsort(batch, kind="stable")
    sb = batch[order]
    counts = np.bincount(batch, minlength=B)
    lo = np.zeros(B + 1, np.int64)
    lo[1:] = np.cumsum(counts)
    core_lo = lo[::SPC]                       # [NC+1]
    cnts = np.diff(core_lo)
    cap = int(np.ceil(max(int(cnts.max()), SUP) / float(SUP)) * SUP)
    NT = cap // 128

    q = global_query.astype(np.float64) @ Wq.T.astype(np.float64) + bq
    # A[h, b, d] = q_h[b] . Wk_h[:, d]   (bk dropped: per-row shift cancels
    # in softmax)
    A = np.einsum("bhk,hkd->hbd", q.reshape(B, H, DK),
                  Wk.reshape(H, DK, N2).astype(np.float64)).astype(np.float32)

    keyT = np.ascontiguousarray(local_key[order].T)   # [N2, E]
    vals = local_value[order]                         # [E, N2]
    sbf = sb.astype(np.float32)

    in_maps = []
    for c in range(NC):
        s, e = int(core_lo[c]), int(core_lo[c + 1])
        n = e - s
        ktc = np.zeros((N2, cap), np.float32)
        ktc[:, :n] = keyT[:, s:e]
        vc = np.zeros((cap, N2), np.float32)
        vc[:n] = vals[s:e]
        vca = np.concatenate(
            [vc, np.ones((cap, 1), np.float32),
             np.zeros((cap, 1), np.float32)], axis=1)
        vcp = np.ascontiguousarray(
            vca.reshape(NT, 128, N2 + 2).transpose(1, 0, 2))
        bc = np.full(cap, -1.0, np.float32)
        bc[:n] = sbf[s:e]
        bcp = np.ascontiguousarray(bc.reshape(NT, 128).T)
        segs = np.arange(c * SPC, (c + 1) * SPC, dtype=np.float32)
        bvrow = np.tile(segs, H)              # [HB], col h*SPC+j
        bvc = np.ascontiguousarray(
            np.broadcast_to(bvrow[None, :], (128, HB)))
        Ac = A[:, c * SPC:(c + 1) * SPC, :]   # [H, SPC, N2]
        atc = np.ascontiguousarray(Ac.reshape(HB, N2).T).astype(np.float32)
        in_maps.append({"kt": ktc, "valp": vcp, "batchf": bcp,
                        "bvals": bvc, "at": atc})

    return in_maps, order, lo, core_lo, cap


def kernel(**inputs):
    gq = np.asarray(inputs["global_query"], np.float32)
    key = np.asarray(inputs["local_key"], np.float32)
    value = np.asarray(inputs["local_value"], np.float32)
    batch = np.asarray(inputs["batch"])
    Wq = np.asarray(inputs["Wq"], np.float32)
    bq = np.asarray(inputs["bq"], np.float32)
    Wk = np.asarray(inputs["Wk"], np.float32)
    bk = np.asarray(inputs["bk"], np.float32)
    Wv = np.asarray(inputs["Wv"], np.float32)
    bv = np.asarray(inputs["bv"], np.float32)
    Wo = np.asarray(inputs["Wo"], np.float32)
    bo = np.asarray(inputs["bo"], np.float32)

    in_maps, order, lo, core_lo, cap = _prep(
        gq, key, value, batch, Wq, bq, Wk, bk, Wv, bv, Wo, bo)
    NT = cap // 128

    nc = _get_kernel(cap)
    res = run_bass_kernel_spmd(nc, in_maps, list(range(NC))).results

    attn = np.zeros((H, B, E), np.float32)
    xcat = np.zeros((B, N2), np.float32)
    WvH = Wv.reshape(H, DK, N2)
    bvH = bv.reshape(H, DK)

    for c in range(NC):
        aup = res[c]["attn_un"]               # [128, NT, HB]
        au = aup.transpose(1, 0, 2).reshape(cap, HB)
        u = res[c]["u_out"]                   # [HB, N2+1]
        gsum = u[:, N2]                       # [HB]
        nz = (gsum > 0).astype(np.float32)
        gsafe = np.where(gsum > 0, gsum, 1.0)
        Un = u[:, :N2] / gsafe[:, None]       # [HB, N2]

        s0 = int(core_lo[c])
        for j in range(SPC):
            b = c * SPC + j
            gs, ge = int(lo[b]), int(lo[b + 1])
            if ge == gs:
                continue
            eidx = order[gs:ge]
            rows = slice(gs - s0, ge - s0)
            valsb = au[rows, j::SPC]          # [cnt_b, H]
            inv = 1.0 / gsafe[j::SPC]         # [H]
            attn[:, b, eidx] = (valsb * inv[None, :]).T

        # out[h, j, dk] = Un[h*SPC+j] @ Wv_h.T + bv_h * (gsum>0)
        UnH = Un.reshape(H, SPC, N2)
        nzH = nz.reshape(H, SPC)
        o = np.einsum("hjd,hkd->hjk", UnH, WvH) + \
            bvH[:, None, :] * nzH[:, :, None]
        xcat[c * SPC:(c + 1) * SPC] = o.transpose(1, 0, 2).reshape(SPC, N2)

    x = (xcat @ Wo.T + bo).astype(np.float32)
    return x, attn


# revision 10
# speedup vs baseline: 1.3178x; 1.0853x over previous
"""Segment-sharded multi-head attention (GNN message passing) for 8 trn2 cores.

Problem: B=64 global queries, E=65536 edges, N2=256, H=8 heads.
reference returns (x [B,N2], attn [H,B,E]); attn is ~98.5% structural zeros
(each edge only attends within its own segment b=batch[e]).

Sharding strategy (per the "shard by segment" hint): sort edges by batch id
on the host (that IS the sharding step), give each core 8 consecutive
segments (~8192 edges). Every softmax row (h, b) is then fully local to one
core -> no cross-device reduction at all, and each core computes only the
nonzero [64 x ~8192] block of attn.

Device kernel per core (CAP = padded edge count):
  scoresT[e, hb] = sum_d key[e,d] * A[hb,d]      (A = q @ Wk_head, host-folded)
  explocal = exp(scoresT) * segmask              (softmax max-shift cancels;
                                                  scores are O(30) so exp is
                                                  safe in f32 without it)
  U_unnorm[hb, 0:256] += explocal.T @ value      (PE accumulation)
  U_unnorm[hb, 256]   += sum_e explocal          (ones column of value)
Host epilogue: attn = explocal / gsum scattered to original edge order,
x = rowwise (U/gsum) @ Wv_h.T (+bv) concat-heads @ Wo.T + bo.

DMA-layout notes: value / batch / attn_un use a partition-major DRAM
packing [128, NT, ...] (edge n*128+p lives at [p, n]) so every DMA burst
is >=2KB contiguous per partition; the segment mask is built on device
from batch ids (is_equal against a per-column b-value table).
"""

import numpy as np

import concourse.bacc as bacc
import concourse.mybir as mybir
import concourse.tile as tile
from concourse.bass_utils import run_bass_kernel_spmd

B = 64          # segments / queries
E = 65536       # edges
H = 8           # heads
DK = 32         # head dim
N2 = 256        # model dim
NC = 8          # cores
SPC = 8         # segments per core
HB = 64         # rows per core = H * SPC
SUP = 1024      # edges per super-tile

F32 = mybir.dt.float32
F32R = mybir.dt.float32r

_kernel_cache: dict[int, object] = {}


def _build_kernel(cap: int):
    """Bass kernel for one core; cap must be a multiple of 512."""
    assert cap % 512 == 0
    NT = cap // 128
    sups = []
    e0 = 0
    while e0 < cap:
        ne = SUP if cap - e0 >= SUP else (cap - e0)
        sups.append((e0, ne))
        e0 += ne

    nc = bacc.Bacc("TRN2", target_bir_lowering=False, debug=False,
                   num_devices=NC)

    kt = nc.dram_tensor("kt", [N2, cap], F32R, kind="ExternalInput")
    valp = nc.dram_tensor("valp", [128, NT, N2 + 2], F32R,
                          kind="ExternalInput")
    batchf = nc.dram_tensor("batchf", [128, NT], F32, kind="ExternalInput")
    bvals = nc.dram_tensor("bvals", [128, HB], F32, kind="ExternalInput")
    at = nc.dram_tensor("at", [N2, HB], F32R, kind="ExternalInput")
    attn_un = nc.dram_tensor("attn_un", [128, NT, HB], F32R,
                             kind="ExternalOutput")
    u_out = nc.dram_tensor("u_out", [HB, N2 + 2], F32,
                            kind="ExternalOutput")

    with tile.TileContext(nc) as tc:
        with (
            tc.tile_pool(name="const", bufs=1) as constp,
            tc.tile_pool(name="ktp", bufs=3) as ktp,
            tc.tile_pool(name="valp_", bufs=3) as valpp,
            tc.tile_pool(name="mskp", bufs=3) as mskp,
            tc.tile_pool(name="expp", bufs=3) as expp,
            tc.tile_pool(name="psp", bufs=3, space="PSUM") as psp,
            tc.tile_pool(name="psu", bufs=1, space="PSUM") as psup,
        ):
            # Constants: A^T (two 128-row contraction chunks), b-value
            # table, per-edge batch ids.
            at_t = constp.tile([128, 2, HB], F32R)
            nc.sync.dma_start(
                at_t[:], at[:, :].rearrange("(c p) h -> p c h", p=128))
            bv_t = constp.tile([128, HB], F32)
            nc.sync.dma_start(bv_t[:], bvals[:, :])
            bat_t = constp.tile([128, NT], F32)
            nc.sync.dma_start(bat_t[:], batchf[:, :])

            u_ps = psup.tile([HB, N2 + 2], F32)

            # Software pipeline: U matmuls for super-tile s are emitted
            # after the scores matmuls of super-tile s+1, so the PE can
            # keep streaming scores while ACT/DVE produce exp(s).
            pending = None        # (exp_t, val_t, nt)
            first_u = True

            def emit_u(p, last):
                nonlocal first_u
                exp_p, val_p, nt_p = p
                for t in range(nt_p):
                    nc.tensor.matmul(u_ps[:], exp_p[:, t, :], val_p[:, t, :],
                                     start=(first_u and t == 0),
                                     stop=(last and t == nt_p - 1),
                                     skip_group_check=True)
                first_u = False

            for (e0, ne) in sups:
                nt = ne // 128
                esl = slice(e0, e0 + ne)
                tsl = slice(e0 // 128, e0 // 128 + nt)

                kt_t = ktp.tile([128, 2, ne], F32R, tag="kt_t")
                nc.sync.dma_start(
                    kt_t[:], kt[:, esl].rearrange("(c p) e -> p c e", p=128))

                val_t = valpp.tile([128, nt, N2 + 2], F32R, tag="val_t")
                nc.sync.dma_start(val_t[:], valp[:, tsl, :])

                # Segment mask from batch ids: msk[p,t,hb] =
                #   (bvals[hb] == batch[edge t*128+p]), built with a
                #   zero-stride broadcast compare.
                msk_t = mskp.tile([128, nt, HB], F32R, tag="msk_t")
                nc.vector.tensor_tensor(
                    msk_t[:],
                    bat_t[:, tsl, None].broadcast_to([128, nt, HB]),
                    bv_t[:, None, :].broadcast_to([128, nt, HB]),
                    mybir.AluOpType.is_equal)

                # scoresT for nt sub-tiles of 128 edges, packed in free dim.
                ps_t = psp.tile([128, nt, HB], F32, tag="ps_t")
                for t in range(nt):
                    ksl = slice(t * 128, (t + 1) * 128)
                    nc.tensor.matmul(ps_t[:, t, :], kt_t[:, 0, ksl],
                                     at_t[:, 0, :], start=True, stop=False,
                                     skip_group_check=True)
                    nc.tensor.matmul(ps_t[:, t, :], kt_t[:, 1, ksl],
                                     at_t[:, 1, :], start=False, stop=True,
                                     skip_group_check=True)

                exp_t = expp.tile([128, nt, HB], F32R, tag="exp_t")
                nc.scalar.activation(exp_t[:], ps_t[:],
                                     mybir.ActivationFunctionType.Exp)
                nc.vector.tensor_mul(exp_t[:], exp_t[:], msk_t[:])

                if pending is not None:
                    emit_u(pending, last=False)
                pending = (exp_t, val_t, nt)

                nc.sync.dma_start(attn_un[:, tsl, :], exp_t[:])

            emit_u(pending, last=True)

            u_sb = constp.tile([HB, N2 + 2], F32)
            nc.vector.tensor_copy(u_sb[:], u_ps[:])
            nc.sync.dma_start(u_out[:], u_sb[:])

    nc.compile()
    return nc


def _get_kernel(cap: int):
    if cap not in _kernel_cache:
        _kernel_cache[cap] = _build_kernel(cap)
    return _kernel_cache[cap]


def _prep(global_query, local_key, local_value, batch,
          Wq, bq, Wk, bk, Wv, bv, Wo, bo):
    order = np.argsort(batch, kind="stable")
    sb = batch[order]
    counts = np.bincount(batch, minlength=B)
    lo = np.zeros(B + 1, np.int64)
    lo[1:] = np.cumsum(counts)
    core_lo = lo[::SPC]                       # [NC+1]
    cnts = np.diff(core_lo)
    cap = int(np.ceil(max(int(cnts.max()), 512) / 512.0) * 512)
    NT = cap // 128

    q = global_query.astype(np.float64) @ Wq.T.astype(np.float64) + bq
    # A[h, b, d] = q_h[b] . Wk_h[:, d]   (bk dropped: per-row shift cancels
    # in softmax)
    A = np.einsum("bhk,hkd->hbd", q.reshape(B, H, DK),
                  Wk.reshape(H, DK, N2).astype(np.float64)).astype(np.float32)

    keyT = np.ascontiguousarray(local_key[order].T)   # [N2, E]
    vals = local_value[order]                         # [E, N2]
    sbf = sb.astype(np.float32)

    in_maps = []
    for c in range(NC):
        s, e = int(core_lo[c]), int(core_lo[c + 1])
        n = e - s
        ktc = np.zeros((N2, cap), np.float32)
        ktc[:, :n] = keyT[:, s:e]
        vc = np.zeros((cap, N2), np.float32)
        vc[:n] = vals[s:e]
        vca = np.concatenate(
            [vc, np.ones((cap, 1), np.float32),
             np.zeros((cap, 1), np.float32)], axis=1)
        vcp = np.ascontiguousarray(
            vca.reshape(NT, 128, N2 + 2).transpose(1, 0, 2))
        bc = np.full(cap, -1.0, np.float32)
        bc[:n] = sbf[s:e]
        bcp = np.ascontiguousarray(bc.reshape(NT, 128).T)
        segs = np.arange(c * SPC, (c + 1) * SPC, dtype=np.float32)
        bvrow = np.tile(segs, H)              # [HB], col h*SPC+j
        bvc = np.ascontiguousarray(
            np.broadcast_to(bvrow[None, :], (128, HB)))
        Ac = A[:, c * SPC:(c + 1) * SPC, :]   # [H, SPC, N2]
        atc = np.ascontiguousarray(Ac.reshape(HB, N2).T).astype(np.float32)
        in_maps.append({"kt": ktc, "valp": vcp, "batchf": bcp,
                        "bvals": bvc, "at": atc})

    return in_maps, order, lo, core_lo, cap


def kernel(**inputs):
    gq = np.asarray(inputs["global_query"], np.float32)
    key = np.asarray(inputs["local_key"], np.float32)
    value = np.asarray(inputs["local_value"], np.float32)
    batch = np.asarray(inputs["batch"])
    Wq = np.asarray(inputs["Wq"], np.float32)
    bq = np.asarray(inputs["bq"], np.float32)
    Wk = np.asarray(inputs["Wk"], np.float32)
    bk = np.asarray(inputs["bk"], np.float32)
    Wv = np.asarray(inputs["Wv"], np.float32)
    bv = np.asarray(inputs["bv"], np.float32)
    Wo = np.asarray(inputs["Wo"], np.float32)
    bo = np.asarray(inputs["bo"], np.float32)

    in_maps, order, lo, core_lo, cap = _prep(
        gq, key, value, batch, Wq, bq, Wk, bk, Wv, bv, Wo, bo)
    NT = cap // 128

    nc = _get_kernel(cap)
    res = run_bass_kernel_spmd(nc, in_maps, list(range(NC))).results

    attn = np.zeros((H, B, E), np.float32)
    xcat = np.zeros((B, N2), np.float32)
    WvH = Wv.reshape(H, DK, N2)
    bvH = bv.reshape(H, DK)

    for c in range(NC):
        aup = res[c]["attn_un"]               # [128, NT, HB]
        au = aup.transpose(1, 0, 2).reshape(cap, HB)
        u = res[c]["u_out"]                   # [HB, N2+1]
        gsum = u[:, N2]                       # [HB]
        nz = (gsum > 0).astype(np.float32)
        gsafe = np.where(gsum > 0, gsum, 1.0)
        Un = u[:, :N2] / gsafe[:, None]       # [HB, N2]

        s0 = int(core_lo[c])
        for j in range(SPC):
            b = c * SPC + j
            gs, ge = int(lo[b]), int(lo[b + 1])
            if ge == gs:
                continue
            eidx = order[gs:ge]
            rows = slice(gs - s0, ge - s0)
            valsb = au[rows, j::SPC]          # [cnt_b, H]
            inv = 1.0 / gsafe[j::SPC]         # [H]
            attn[:, b, eidx] = (valsb * inv[None, :]).T

        # out[h, j, dk] = Un[h*SPC+j] @ Wv_h.T + bv_h * (gsum>0)
        UnH = Un.reshape(H, SPC, N2)
        nzH = nz.reshape(H, SPC)
        o = np.einsum("hjd,hkd->hjk", UnH, WvH) + \
            bvH[:, None, :] * nzH[:, :, None]
        xcat[c * SPC:(c + 1) * SPC] = o.transpose(1, 0, 2).reshape(SPC, N2)

    x = (xcat @ Wo.T + bo).astype(np.float32)
    return x, attn


# revision 25
# speedup vs baseline: 2.3543x; 1.7865x over previous
"""Segment-sharded multi-head attention (GNN message passing) for 8 trn2 cores.

Problem: B=64 global queries, E=65536 edges, N2=256, H=8 heads.
reference returns (x [B,N2], attn [H,B,E]); attn is ~98.5% structural zeros
(each edge only attends within its own segment b=batch[e]).

Sharding strategy (per the "shard by segment" hint): sort edges by batch id
on the host (that IS the sharding step), give each core 8 consecutive
segments (~8192 edges). Every softmax row (h, b) is then fully local to one
core -> no cross-device reduction at all, and each core computes only the
nonzero [64 x ~8192] block of attn.

Device kernel per core (CAP = padded edge count):
  scoresT[e, hb] = sum_d key[e,d] * A[hb,d]      (A = q @ Wk_head, host-folded)
  explocal = exp(scoresT) * segmask              (softmax max-shift cancels;
                                                  scores are O(30) so exp is
                                                  safe without it)
  U_unnorm[hb, 0:256] += explocal.T @ value      (PE accumulation)
  U_unnorm[hb, 256]   += sum_e explocal          (ones column of value)
Host epilogue: attn = explocal / gsum scattered to original edge order,
x = rowwise (U/gsum) @ Wv_h.T (+bv) concat-heads @ Wo.T + bo.

Performance notes (measured on trn2 via ntff traces):
- dtypes: key/A in fp16 (halves the dominant DMA stream; softmax row
  normalization cancels most common-mode score error -> ~1.7e-3 rel-l2),
  value/exp/attn in bf16, PSUM accumulation in fp32.
- value / batch / attn_un use a partition-major DRAM packing [128, NT, .]
  (edge n*128+p lives at [p, n]) so every DMA burst is >=2KB contiguous.
- segment mask built on device via a zero-stride broadcast is_equal.
- DMA queues: kt alone on the Sync HWDGE so score inputs are never stuck
  behind other traffic; val+consts on the Scalar HWDGE; attn stores on
  GpSimd SWDGE.
- software pipeline: U matmuls for piece s are emitted after the scores
  matmuls of piece s+1; variable piece sizes (512 first / 2048 middle /
  <=512 tail) shorten the pipeline head and tail.
"""

import numpy as np
from ml_dtypes import bfloat16

import concourse.bacc as bacc
import concourse.mybir as mybir
import concourse.tile as tile
from concourse.bass_utils import run_bass_kernel_spmd

B = 64          # segments / queries
E = 65536       # edges
H = 8           # heads
DK = 32         # head dim
N2 = 256        # model dim
NC = 8          # cores
SPC = 8         # segments per core
HB = 64         # rows per core = H * SPC
SUP = 2048      # edges per super-tile

F32 = mybir.dt.float32
F32R = mybir.dt.float32r
BF16 = mybir.dt.bfloat16
FP16 = mybir.dt.float16

_kernel_cache: dict[int, object] = {}


def _build_kernel(cap: int):
    """Bass kernel for one core; cap must be a multiple of 512."""
    assert cap % 128 == 0
    NT = cap // 128
    # Variable piece sizes: small first piece so the PE starts early,
    # big middle pieces for DMA/instruction efficiency, small tail so the
    # final scores->exp->U->store chain is short.
    sizes = []
    rem = cap
    first = min(512, rem)
    sizes.append(first)
    rem -= first
    while rem > 0:
        if rem > SUP + 512:
            take = SUP
        elif rem > 512:
            take = rem - 512
        else:
            take = rem
        sizes.append(take)
        rem -= take
    sups = []
    e0 = 0
    for ne in sizes:
        sups.append((e0, ne))
        e0 += ne

    nc = bacc.Bacc("TRN2", target_bir_lowering=False, debug=False,
                   num_devices=NC)

    kt = nc.dram_tensor("kt", [N2, cap], FP16, kind="ExternalInput")
    valp = nc.dram_tensor("valp", [128, NT, N2 + 2], BF16,
                          kind="ExternalInput")
    batchf = nc.dram_tensor("batchf", [128, NT], F32, kind="ExternalInput")
    bvals = nc.dram_tensor("bvals", [128, HB], F32, kind="ExternalInput")
    at = nc.dram_tensor("at", [N2, HB], FP16, kind="ExternalInput")
    attn_un = nc.dram_tensor("attn_un", [128, NT, HB], BF16,
                             kind="ExternalOutput")
    u_out = nc.dram_tensor("u_out", [HB, N2 + 2], F32,
                            kind="ExternalOutput")

    with tile.TileContext(nc) as tc:
        with (
            tc.tile_pool(name="const", bufs=1) as constp,
            tc.tile_pool(name="ktp", bufs=4) as ktp,
            tc.tile_pool(name="valp_", bufs=4) as valpp,
            tc.tile_pool(name="mskp", bufs=3) as mskp,
            tc.tile_pool(name="expp", bufs=4) as expp,
            tc.tile_pool(name="psp", bufs=3, space="PSUM") as psp,
            tc.tile_pool(name="psu", bufs=1, space="PSUM") as psup,
        ):
            # Constants: A^T (two 128-row contraction chunks), b-value
            # table, per-edge batch ids.
            at_t = constp.tile([128, 2, HB], FP16)
            nc.scalar.dma_start(
                at_t[:], at[:, :].rearrange("(c p) h -> p c h", p=128))
            bv_t = constp.tile([128, HB], F32)
            nc.scalar.dma_start(bv_t[:], bvals[:, :])
            bat_t = constp.tile([128, NT], F32)
            nc.scalar.dma_start(bat_t[:], batchf[:, :])

            u_ps = psup.tile([HB, N2 + 2], F32)

            # Software pipeline: U matmuls for super-tile s are emitted
            # after the scores matmuls of super-tile s+1, so the PE can
            # keep streaming scores while ACT/DVE produce exp(s).
            pending = None        # (exp_t, val_t, nt)
            first_u = True

            def emit_u(p, last):
                nonlocal first_u
                exp_p, val_p, nt_p = p
                for t in range(nt_p):
                    nc.tensor.matmul(u_ps[:], exp_p[:, t, :], val_p[:, t, :],
                                     start=(first_u and t == 0),
                                     stop=(last and t == nt_p - 1),
                                     skip_group_check=True)
                first_u = False

            for pi, (e0, ne) in enumerate(sups):
                nt = ne // 128
                esl = slice(e0, e0 + ne)
                tsl = slice(e0 // 128, e0 // 128 + nt)

                kt_t = ktp.tile([128, 2, ne], FP16, tag="kt_t")
                nc.sync.dma_start(
                    kt_t[:], kt[:, esl].rearrange("(c p) e -> p c e", p=128))

                val_t = valpp.tile([128, nt, N2 + 2], BF16, tag="val_t")
                nc.scalar.dma_start(val_t[:], valp[:, tsl, :])

                # Segment mask from batch ids: msk[p,t,hb] =
                #   (bvals[hb] == batch[edge t*128+p]), built with a
                #   zero-stride broadcast compare.
                msk_t = mskp.tile([128, nt, HB], BF16, tag="msk_t")
                nc.vector.tensor_tensor(
                    msk_t[:],
                    bat_t[:, tsl, None].broadcast_to([128, nt, HB]),
                    bv_t[:, None, :].broadcast_to([128, nt, HB]),
                    mybir.AluOpType.is_equal)

                # scoresT for nt sub-tiles of 128 edges, packed in free dim.
                ps_t = psp.tile([128, nt, HB], F32, tag="ps_t")
                for t in range(nt):
                    ksl = slice(t * 128, (t + 1) * 128)
                    nc.tensor.matmul(ps_t[:, t, :], kt_t[:, 0, ksl],
                                     at_t[:, 0, :], start=True, stop=False,
                                     skip_group_check=True)
                    nc.tensor.matmul(ps_t[:, t, :], kt_t[:, 1, ksl],
                                     at_t[:, 1, :], start=False, stop=True,
                                     skip_group_check=True)

                exp_t = expp.tile([128, nt, HB], BF16, tag="exp_t")
                nc.scalar.activation(exp_t[:], ps_t[:],
                                     mybir.ActivationFunctionType.Exp)
                nc.vector.tensor_mul(exp_t[:], exp_t[:], msk_t[:])

                if pending is not None:
                    emit_u(pending, last=False)
                pending = (exp_t, val_t, nt)

                nc.gpsimd.dma_start(attn_un[:, tsl, :], exp_t[:])

            emit_u(pending, last=True)

            u_sb = constp.tile([HB, N2 + 2], F32)
            nc.vector.tensor_copy(u_sb[:], u_ps[:])
            nc.gpsimd.dma_start(u_out[:], u_sb[:])

    nc.compile()
    return nc


def _get_kernel(cap: int):
    if cap not in _kernel_cache:
        _kernel_cache[cap] = _build_kernel(cap)
    return _kernel_cache[cap]


def _prep(global_query, local_key, local_value, batch,
          Wq, bq, Wk, bk, Wv, bv, Wo, bo):
    order = np.argsort(batch, kind="stable")
    sb = batch[order]
    counts = np.bincount(batch, minlength=B)
    lo = np.zeros(B + 1, np.int64)
    lo[1:] = np.cumsum(counts)
    core_lo = lo[::SPC]                       # [NC+1]
    cnts = np.diff(core_lo)
    cap = int(np.ceil(max(int(cnts.max()), 512) / 128.0) * 128)
    NT = cap // 128

    q = global_query.astype(np.float64) @ Wq.T.astype(np.float64) + bq
    # A[h, b, d] = q_h[b] . Wk_h[:, d]   (bk dropped: per-row shift cancels
    # in softmax)
    A = np.einsum("bhk,hkd->hbd", q.reshape(B, H, DK),
                  Wk.reshape(H, DK, N2).astype(np.float64)).astype(np.float32)

    keyT = np.ascontiguousarray(local_key[order].T)   # [N2, E]
    vals = local_value[order]                         # [E, N2]
    sbf = sb.astype(np.float32)

    in_maps = []
    for c in range(NC):
        s, e = int(core_lo[c]), int(core_lo[c + 1])
        n = e - s
        ktc = np.zeros((N2, cap), np.float16)
        ktc[:, :n] = keyT[:, s:e].astype(np.float16)
        vc = np.zeros((cap, N2), np.float32)
        vc[:n] = vals[s:e]
        vca = np.concatenate(
            [vc, np.ones((cap, 1), np.float32),
             np.zeros((cap, 1), np.float32)], axis=1)
        vcp = np.ascontiguousarray(
            vca.reshape(NT, 128, N2 + 2).transpose(1, 0, 2)).astype(bfloat16)
        bc = np.full(cap, -1.0, np.float32)
        bc[:n] = sbf[s:e]
        bcp = np.ascontiguousarray(bc.reshape(NT, 128).T)
        segs = np.arange(c * SPC, (c + 1) * SPC, dtype=np.float32)
        bvrow = np.tile(segs, H)              # [HB], col h*SPC+j
        bvc = np.ascontiguousarray(
            np.broadcast_to(bvrow[None, :], (128, HB)))
        Ac = A[:, c * SPC:(c + 1) * SPC, :]   # [H, SPC, N2]
        atc = np.ascontiguousarray(Ac.reshape(HB, N2).T).astype(np.float16)
        in_maps.append({"kt": ktc, "valp": vcp, "batchf": bcp,
                        "bvals": bvc, "at": atc})

    return in_maps, order, lo, core_lo, cap


def kernel(**inputs):
    gq = np.asarray(inputs["global_query"], np.float32)
    key = np.asarray(inputs["local_key"], np.float32)
    value = np.asarray(inputs["local_value"], np.float32)
    batch = np.asarray(inputs["batch"])
    Wq = np.asarray(inputs["Wq"], np.float32)
    bq = np.asarray(inputs["bq"], np.float32)
    Wk = np.asarray(inputs["Wk"], np.float32)
    bk = np.asarray(inputs["bk"], np.float32)
    Wv = np.asarray(inputs["Wv"], np.float32)
    bv = np.asarray(inputs["bv"], np.float32)
    Wo = np.asarray(inputs["Wo"], np.float32)
    bo = np.asarray(inputs["bo"], np.float32)

    in_maps, order, lo, core_lo, cap = _prep(
        gq, key, value, batch, Wq, bq, Wk, bk, Wv, bv, Wo, bo)
    NT = cap // 128

    nc = _get_kernel(cap)
    res = run_bass_kernel_spmd(nc, in_maps, list(range(NC))).results

    attn = np.zeros((H, B, E), np.float32)
    xcat = np.zeros((B, N2), np.float32)
    WvH = Wv.reshape(H, DK, N2)
    bvH = bv.reshape(H, DK)

    for c in range(NC):
        aup = res[c]["attn_un"]               # [128, NT, HB] bf16
        au = aup.transpose(1, 0, 2).reshape(cap, HB).astype(np.float32)
        u = res[c]["u_out"]                   # [HB, N2+1]
        gsum = u[:, N2]                       # [HB]
        nz = (gsum > 0).astype(np.float32)
        gsafe = np.where(gsum > 0, gsum, 1.0)
        Un = u[:, :N2] / gsafe[:, None]       # [HB, N2]

        s0 = int(core_lo[c])
        for j in range(SPC):
            b = c * SPC + j
            gs, ge = int(lo[b]), int(lo[b + 1])
            if ge == gs:
                continue
            eidx = order[gs:ge]
            rows = slice(gs - s0, ge - s0)
            valsb = au[rows, j::SPC]          # [cnt_b, H]
            inv = 1.0 / gsafe[j::SPC]         # [H]
            attn[:, b, eidx] = (valsb * inv[None, :]).T

        # out[h, j, dk] = Un[h*SPC+j] @ Wv_h.T + bv_h * (gsum>0)
        UnH = Un.reshape(H, SPC, N2)
        nzH = nz.reshape(H, SPC)
        o = np.einsum("hjd,hkd->hjk", UnH, WvH) + \
            bvH[:, None, :] * nzH[:, :, None]
        xcat[c * SPC:(c + 1) * SPC] = o.transpose(1, 0, 2).reshape(SPC, N2)

    x = (xcat @ Wo.T + bo).astype(np.float32)
    return x, attn


# revision 29
# speedup vs baseline: 2.7299x; 1.1595x over previous
"""Segment-sharded multi-head attention (GNN message passing) for 8 trn2 cores.

Problem: B=64 global queries, E=65536 edges, N2=256, H=8 heads.
reference returns (x [B,N2], attn [H,B,E]); attn is ~98.5% structural zeros
(each edge only attends within its own segment b=batch[e]).

Sharding strategy (per the "shard by segment" hint): sort edges by batch id
on the host (that IS the sharding step), give each core 8 consecutive
segments (~8192 edges). Every softmax row (h, b) is then fully local to one
core -> no cross-device reduction at all, and each core computes only the
nonzero [64 x ~8192] block of attn.

Device kernel per core (CAP = padded edge count):
  scoresT[e, hb] = sum_d key[e,d] * A[hb,d]      (A = q @ Wk_head, host-folded)
  explocal = exp(scoresT) * segmask              (softmax max-shift cancels;
                                                  scores are O(30) so exp is
                                                  safe without it)
  U_unnorm[hb, 0:256] += explocal.T @ value      (PE accumulation)
  U_unnorm[hb, 256]   += sum_e explocal          (ones column of value)
Host epilogue: attn = explocal / gsum scattered to original edge order,
x = rowwise (U/gsum) @ Wv_h.T (+bv) concat-heads @ Wo.T + bo.

Performance notes (measured on trn2 via ntff traces):
- dtypes: key/A in fp16 (halves the dominant DMA stream; softmax row
  normalization cancels most common-mode score error -> ~1.7e-3 rel-l2),
  value/exp/attn in bf16, PSUM accumulation in fp32.
- value / batch / attn_un use a partition-major DRAM packing [128, NT, .]
  (edge n*128+p lives at [p, n]) so every DMA burst is >=2KB contiguous.
- segment mask built on device via a zero-stride broadcast is_equal.
- DMA queues: kt alone on the Sync HWDGE so score inputs are never stuck
  behind other traffic; val+consts on the Scalar HWDGE; attn stores on
  GpSimd SWDGE.
- software pipeline: U matmuls for piece s are emitted after the scores
  matmuls of piece s+1; variable piece sizes (512 first / 2048 middle /
  <=512 tail) shorten the pipeline head and tail.
"""

import numpy as np
from ml_dtypes import bfloat16

import concourse.bacc as bacc
import concourse.mybir as mybir
import concourse.tile as tile
from concourse.bass_utils import run_bass_kernel_spmd

B = 64          # segments / queries
E = 65536       # edges
H = 8           # heads
DK = 32         # head dim
N2 = 256        # model dim
NC = 8          # cores
SPC = 8         # segments per core
HB = 64         # rows per core = H * SPC
SUP = 2048      # edges per super-tile

F32 = mybir.dt.float32
F32R = mybir.dt.float32r
BF16 = mybir.dt.bfloat16
FP16 = mybir.dt.float16

_kernel_cache: dict[int, object] = {}


def _build_kernel(cap: int):
    """Bass kernel for one core; cap must be a multiple of 512."""
    assert cap % 128 == 0
    NT = cap // 128
    # Variable piece sizes: small first piece so the PE starts early,
    # big middle pieces for DMA/instruction efficiency, small tail so the
    # final scores->exp->U->store chain is short.
    sizes = []
    rem = cap
    first = min(512, rem)
    sizes.append(first)
    rem -= first
    while rem > 0:
        if rem > SUP + 512:
            take = SUP
        elif rem > 512:
            take = rem - 512
        else:
            take = rem
        sizes.append(take)
        rem -= take
    sups = []
    e0 = 0
    for ne in sizes:
        sups.append((e0, ne))
        e0 += ne

    nc = bacc.Bacc("TRN2", target_bir_lowering=False, debug=False,
                   num_devices=NC)

    kt = nc.dram_tensor("kt", [N2, cap], FP16, kind="ExternalInput")
    valp = nc.dram_tensor("valp", [128, NT, N2 + 2], BF16,
                          kind="ExternalInput")
    batchf = nc.dram_tensor("batchf", [128, NT], F32, kind="ExternalInput")
    bvals = nc.dram_tensor("bvals", [128, HB], F32, kind="ExternalInput")
    at = nc.dram_tensor("at", [N2, HB], FP16, kind="ExternalInput")
    attn_un = nc.dram_tensor("attn_un", [128, NT, HB], BF16,
                             kind="ExternalOutput")
    u_out = nc.dram_tensor("u_out", [HB, N2 + 2], F32,
                            kind="ExternalOutput")

    with tile.TileContext(nc) as tc:
        with (
            tc.tile_pool(name="const", bufs=1) as constp,
            tc.tile_pool(name="ktp", bufs=4) as ktp,
            tc.tile_pool(name="valp_", bufs=4) as valpp,
            tc.tile_pool(name="mskp", bufs=3) as mskp,
            tc.tile_pool(name="expp", bufs=4) as expp,
            tc.tile_pool(name="psp", bufs=3, space="PSUM") as psp,
            tc.tile_pool(name="psu", bufs=1, space="PSUM") as psup,
        ):
            # Constants: A^T (two 128-row contraction chunks), b-value
            # table, per-edge batch ids.
            at_t = constp.tile([128, 2, HB], FP16)
            nc.scalar.dma_start(
                at_t[:], at[:, :].rearrange("(c p) h -> p c h", p=128))
            bv_t = constp.tile([128, HB], F32)
            nc.scalar.dma_start(bv_t[:], bvals[:, :])
            bat_t = constp.tile([128, NT], F32)
            nc.scalar.dma_start(bat_t[:], batchf[:, :])

            u_ps = psup.tile([HB, N2 + 2], F32)

            # Software pipeline: U matmuls for super-tile s are emitted
            # after the scores matmuls of super-tile s+1, so the PE can
            # keep streaming scores while ACT/DVE produce exp(s).
            pending = None        # (exp_t, val_t, nt)
            first_u = True

            def emit_u(p, last):
                nonlocal first_u
                exp_p, val_p, nt_p = p
                for t in range(nt_p):
                    nc.tensor.matmul(u_ps[:], exp_p[:, t, :], val_p[:, t, :],
                                     start=(first_u and t == 0),
                                     stop=(last and t == nt_p - 1),
                                     skip_group_check=True)
                first_u = False

            for pi, (e0, ne) in enumerate(sups):
                nt = ne // 128
                esl = slice(e0, e0 + ne)
                tsl = slice(e0 // 128, e0 // 128 + nt)

                kt_t = ktp.tile([128, 2, ne], FP16, tag="kt_t")
                nc.sync.dma_start(
                    kt_t[:], kt[:, esl].rearrange("(c p) e -> p c e", p=128))

                val_t = valpp.tile([128, nt, N2 + 2], BF16, tag="val_t")
                nc.scalar.dma_start(val_t[:], valp[:, tsl, :])

                # Segment mask from batch ids: msk[p,t,hb] =
                #   (bvals[hb] == batch[edge t*128+p]), built with a
                #   zero-stride broadcast compare.
                msk_t = mskp.tile([128, nt, HB], BF16, tag="msk_t")
                nc.vector.tensor_tensor(
                    msk_t[:],
                    bat_t[:, tsl, None].broadcast_to([128, nt, HB]),
                    bv_t[:, None, :].broadcast_to([128, nt, HB]),
                    mybir.AluOpType.is_equal)

                # scoresT for nt sub-tiles of 128 edges, packed in free dim.
                ps_t = psp.tile([128, nt, HB], F32, tag="ps_t")
                for t in range(nt):
                    ksl = slice(t * 128, (t + 1) * 128)
                    nc.tensor.matmul(ps_t[:, t, :], kt_t[:, 0, ksl],
                                     at_t[:, 0, :], start=True, stop=False,
                                     skip_group_check=True)
                    nc.tensor.matmul(ps_t[:, t, :], kt_t[:, 1, ksl],
                                     at_t[:, 1, :], start=False, stop=True,
                                     skip_group_check=True)

                exp_t = expp.tile([128, nt, HB], BF16, tag="exp_t")
                nc.scalar.activation(exp_t[:], ps_t[:],
                                     mybir.ActivationFunctionType.Exp)
                nc.vector.tensor_mul(exp_t[:], exp_t[:], msk_t[:])

                if pending is not None:
                    emit_u(pending, last=False)
                pending = (exp_t, val_t, nt)

                nc.gpsimd.dma_start(attn_un[:, tsl, :], exp_t[:])

            emit_u(pending, last=True)

            u_sb = constp.tile([HB, N2 + 2], F32)
            nc.vector.tensor_copy(u_sb[:], u_ps[:])
            nc.gpsimd.dma_start(u_out[:], u_sb[:])

    nc.compile()
    return nc


def _get_kernel(cap: int):
    if cap not in _kernel_cache:
        _kernel_cache[cap] = _build_kernel(cap)
    return _kernel_cache[cap]


def _prep(global_query, local_key, local_value, batch,
          Wq, bq, Wk, bk, Wv, bv, Wo, bo):
    order = np.argsort(batch, kind="stable")
    sb = batch[order]
    counts = np.bincount(batch, minlength=B)
    lo = np.zeros(B + 1, np.int64)
    lo[1:] = np.cumsum(counts)
    core_lo = lo[::SPC]                       # [NC+1]
    cnts = np.diff(core_lo)
    cap = int(np.ceil(max(int(cnts.max()), 512) / 128.0) * 128)
    NT = cap // 128

    q = global_query.astype(np.float64) @ Wq.T.astype(np.float64) + bq
    # A[h, b, d] = q_h[b] . Wk_h[:, d]   (bk dropped: per-row shift cancels
    # in softmax)
    A = np.einsum("bhk,hkd->hbd", q.reshape(B, H, DK),
                  Wk.reshape(H, DK, N2).astype(np.float64)).astype(np.float32)

    keyT = np.ascontiguousarray(local_key[order].T)   # [N2, E]
    vals = local_value[order]                         # [E, N2]
    sbf = sb.astype(np.float32)

    in_maps = []
    for c in range(NC):
        s, e = int(core_lo[c]), int(core_lo[c + 1])
        n = e - s
        ktc = np.zeros((N2, cap), np.float16)
        ktc[:, :n] = keyT[:, s:e].astype(np.float16)
        vc = np.zeros((cap, N2), np.float32)
        vc[:n] = vals[s:e]
        vca = np.concatenate(
            [vc, np.ones((cap, 1), np.float32),
             np.zeros((cap, 1), np.float32)], axis=1)
        vcp = np.ascontiguousarray(
            vca.reshape(NT, 128, N2 + 2).transpose(1, 0, 2)).astype(bfloat16)
        bc = np.full(cap, -1.0, np.float32)
        bc[:n] = sbf[s:e]
        bcp = np.ascontiguousarray(bc.reshape(NT, 128).T)
        segs = np.arange(c * SPC, (c + 1) * SPC, dtype=np.float32)
        bvrow = np.tile(segs, H)              # [HB], col h*SPC+j
        bvc = np.ascontiguousarray(
            np.broadcast_to(bvrow[None, :], (128, HB)))
        Ac = A[:, c * SPC:(c + 1) * SPC, :]   # [H, SPC, N2]
        atc = np.ascontiguousarray(Ac.reshape(HB, N2).T).astype(np.float16)
        in_maps.append({"kt": ktc, "valp": vcp, "batchf": bcp,
                        "bvals": bvc, "at": atc})

    return in_maps, order, lo, core_lo, cap


def kernel(**inputs):
    gq = np.asarray(inputs["global_query"], np.float32)
    key = np.asarray(inputs["local_key"], np.float32)
    value = np.asarray(inputs["local_value"], np.float32)
    batch = np.asarray(inputs["batch"])
    Wq = np.asarray(inputs["Wq"], np.float32)
    bq = np.asarray(inputs["bq"], np.float32)
    Wk = np.asarray(inputs["Wk"], np.float32)
    bk = np.asarray(inputs["bk"], np.float32)
    Wv = np.asarray(inputs["Wv"], np.float32)
    bv = np.asarray(inputs["bv"], np.float32)
    Wo = np.asarray(inputs["Wo"], np.float32)
    bo = np.asarray(inputs["bo"], np.float32)

    in_maps, order, lo, core_lo, cap = _prep(
        gq, key, value, batch, Wq, bq, Wk, bk, Wv, bv, Wo, bo)
    NT = cap // 128

    nc = _get_kernel(cap)
    res = run_bass_kernel_spmd(nc, in_maps, list(range(NC))).results

    attn = np.zeros((H, B, E), np.float32)
    xcat = np.zeros((B, N2), np.float32)
    WvH = Wv.reshape(H, DK, N2)
    bvH = bv.reshape(H, DK)

    for c in range(NC):
        aup = res[c]["attn_un"]               # [128, NT, HB] bf16
        au = aup.transpose(1, 0, 2).reshape(cap, HB).astype(np.float32)
        u = res[c]["u_out"]                   # [HB, N2+1]
        gsum = u[:, N2]                       # [HB]
        nz = (gsum > 0).astype(np.float32)
        gsafe = np.where(gsum > 0, gsum, 1.0)
        Un = u[:, :N2] / gsafe[:, None]       # [HB, N2]

        s0 = int(core_lo[c])
        for j in range(SPC):
            b = c * SPC + j
            gs, ge = int(lo[b]), int(lo[b + 1])
            if ge == gs:
                continue
            eidx = order[gs:ge]
            rows = slice(gs - s0, ge - s0)
            valsb = au[rows, j::SPC]          # [cnt_b, H]
            inv = 1.0 / gsafe[j::SPC]         # [H]
            attn[:, b, eidx] = (valsb * inv[None, :]).T

        # out[h, j, dk] = Un[h*SPC+j] @ Wv_h.T + bv_h * (gsum>0)
        UnH = Un.reshape(H, SPC, N2)
        nzH = nz.reshape(H, SPC)
        o = np.einsum("hjd,hkd->hjk", UnH, WvH) + \
            bvH[:, None, :] * nzH[:, :, None]
        xcat[c * SPC:(c + 1) * SPC] = o.transpose(1, 0, 2).reshape(SPC, N2)

    x = (xcat @ Wo.T + bo).astype(np.float32)
    return x, attn
